# revision 3
# baseline (speedup 1.0000x reference)
"""Trainium2 Bass kernel for GPyTorch-style RBF-kernel features + linear head.

Reference computation (per full input):
    xs = x.reshape(BL, D) / lengthscale
    cs = centers / lengthscale
    sq = |xs|^2[:,None] + |cs|^2[None,:] - 2 xs @ cs.T
    K  = exp(-0.5 * max(sq, 0))
    out = K @ W_out.T + b_out

Strategy (8-core data parallel over rows; centers/lengthscale/W/b replicated):
  x is pre-transposed and cast to bf16 on the host (layout prep only) so the
  device streams [d-on-partitions, m-free] tiles straight into the PE array —
  no on-chip transposes. Per core (M=4096 rows) in 8 blocks of 512 rows m:
    xT    : x.T block [128, dc(4) x 512] bf16   (one DMA)
    xT2   : xT*xT                               (DVE, bf16 SBUF fast mode)
    nr    : sum_d (-0.5*invl2[d]) xT2[d,m] -> PSUM [1,512]  (PE, 4 matmuls)
    fr    : bf16(nr)                            (DVE copy)
    S     : sum_dc csT2.T @ xT  + ones x fr rank-1 fold   (PE, 5 matmuls/tile)
    E     : exp(S + cnh[n]) -> bf16             (ACT, per-partition bias)
    G     : sum_nt wT.T @ E                     (PE, 4 matmuls/do-tile)
    out   : Identity(G) + b_out[do] -> bf16     (ACT, per-partition bias)
  The row-norm enters the exponent additively pre-exp (rank-1 fold), so E is
  exactly the RBF kernel value in [0,1] — bf16-safe.  Output is produced
  transposed per block and untransposed + cast to f32 on the host.
"""

import sys
import types

import numpy as np
import ml_dtypes

# The container's axon build lacks the NTFF profile hook module that
# bass_utils imports when trace=True; shim it so imports never fail.
_shim = types.ModuleType("antenv.axon_hooks")
_shim.get_axon_ntff_profile_hook = lambda: None
sys.modules.setdefault("antenv.axon_hooks", _shim)

import concourse.bacc as bacc
import concourse.tile as tile
from concourse import mybir

N_CORES = 8
B, L, D = 4, 8192, 512
BL = B * L
M_CORE = BL // N_CORES          # 4096 rows per core
MB = 512                        # rows per block
N_BLOCKS = M_CORE // MB         # 8
NT = D // 128                   # 4 chunks along any 512 dim

F32 = mybir.dt.float32
BF16 = mybir.dt.bfloat16


def build_nc(n_blocks=N_BLOCKS, loop_repeat=1):
    nc = bacc.Bacc("TRN2", debug=False, num_devices=N_CORES)

    xt_d = nc.dram_tensor("xt", [128, n_blocks * NT * MB], BF16, kind="ExternalInput").ap()
    cs_d = nc.dram_tensor("csT2", [128, NT * NT * 128], BF16, kind="ExternalInput").ap()
    wt_d = nc.dram_tensor("wT", [128, NT * NT * 128], BF16, kind="ExternalInput").ap()
    cnh_d = nc.dram_tensor("cnh", [128, NT], F32, kind="ExternalInput").ap()
    bcol_d = nc.dram_tensor("bcol", [128, NT], F32, kind="ExternalInput").ap()
    negiv_d = nc.dram_tensor("negiv", [128, NT], BF16, kind="ExternalInput").ap()
    onesb_d = nc.dram_tensor("onesb", [1, 128], BF16, kind="ExternalInput").ap()
    y_d = nc.dram_tensor("y", [n_blocks, 128, NT * MB], BF16, kind="ExternalOutput").ap()

    with tile.TileContext(nc) as tc:
        with (
            tc.tile_pool(name="consts", bufs=1) as cp,
            tc.tile_pool(name="xin", bufs=3) as xp,
            tc.tile_pool(name="xsq", bufs=2) as xqp,
            tc.tile_pool(name="frp", bufs=2) as frp,
            tc.tile_pool(name="ework", bufs=6) as ep,
            tc.tile_pool(name="oout", bufs=2) as op,
            tc.tile_pool(name="ps_s", bufs=4, space="PSUM") as pss,
            tc.tile_pool(name="ps_o", bufs=3, space="PSUM") as pso,
            tc.tile_pool(name="ps_r", bufs=1, space="PSUM") as psr,
        ):
            # ---- constants into SBUF (once) ----
            csT2 = cp.tile([128, NT * NT * 128], BF16, tag="csT2")
            nc.sync.dma_start(csT2[:], cs_d[:])
            wT = cp.tile([128, NT * NT * 128], BF16, tag="wT")
            nc.sync.dma_start(wT[:], wt_d[:])
            cnh = cp.tile([128, NT], F32, tag="cnh")
            nc.sync.dma_start(cnh[:], cnh_d[:])
            bcol = cp.tile([128, NT], F32, tag="bcol")
            nc.sync.dma_start(bcol[:], bcol_d[:])
            negiv = cp.tile([128, NT], BF16, tag="negiv")
            nc.sync.dma_start(negiv[:], negiv_d[:])
            onesb = cp.tile([1, 128], BF16, tag="onesb")
            nc.sync.dma_start(onesb[:], onesb_d[:])

            def block(mb):
                # ---- load xT block: [128(d%128), (dc m)] bf16 ----
                xT = xp.tile([128, NT * MB], BF16, tag="xT")
                nc.sync.dma_start(
                    xT[:], xt_d[:, mb * NT * MB:(mb + 1) * NT * MB]
                )

                # ---- xT2 = xT*xT (bf16, SBUF-only: DVE fast mode) ----
                xT2 = xqp.tile([128, NT * MB], BF16, tag="xT2")
                nc.vector.tensor_tensor(
                    xT2[:], xT[:], xT[:], mybir.AluOpType.mult
                )

                # ---- nr[1, m] = sum_d (-0.5*invl2[d]) * xT2[d, m] ----
                nr_ps = psr.tile([1, MB], F32, tag="nr")
                for dc in range(NT):
                    nc.tensor.matmul(
                        nr_ps[:],
                        negiv[:, dc:dc + 1],
                        xT2[:, dc * MB:(dc + 1) * MB],
                        start=(dc == 0),
                        stop=(dc == NT - 1),
                    )
                fr = frp.tile([1, MB], BF16, tag="fr")
                nc.vector.tensor_copy(fr[:], nr_ps[:])

                # ---- mm1 + rank-1 row-norm fold + exp per n-tile ----
                e_tiles = []
                for nt in range(NT):
                    s_ps = pss.tile([128, MB], F32, tag="ps")
                    for dc in range(NT):
                        nc.tensor.matmul(
                            s_ps[:],
                            csT2[:, (dc * NT + nt) * 128:(dc * NT + nt + 1) * 128],
                            xT[:, dc * MB:(dc + 1) * MB],
                            start=(dc == 0),
                            stop=False,
                        )
                    nc.tensor.matmul(
                        s_ps[:],
                        onesb[:],
                        fr[:],
                        start=False,
                        stop=True,
                        skip_group_check=True,
                    )
                    e_t = ep.tile([128, MB], BF16, tag="e")
                    nc.scalar.activation(
                        e_t[:], s_ps[:], mybir.ActivationFunctionType.Exp,
                        bias=cnh[:, nt:nt + 1], scale=1.0,
                    )
                    e_tiles.append(e_t)

                # ---- mm2 (dot-outer) + bias via ACT Identity ----
                out_sb = op.tile([128, NT * MB], BF16, tag="osb")
                for dot in range(NT):
                    o_ps = pso.tile([128, MB], F32, tag="po")
                    for nt in range(NT):
                        nc.tensor.matmul(
                            o_ps[:],
                            wT[:, (nt * NT + dot) * 128:(nt * NT + dot + 1) * 128],
                            e_tiles[nt][:],
                            start=(nt == 0),
                            stop=(nt == NT - 1),
                        )
                    nc.scalar.activation(
                        out_sb[:, dot * MB:(dot + 1) * MB], o_ps[:],
                        mybir.ActivationFunctionType.Identity,
                        bias=bcol[:, dot:dot + 1], scale=1.0,
                    )

                nc.sync.dma_start(y_d[mb], out_sb[:])

            def body():
                for mb in range(n_blocks):
                    block(mb)

            if loop_repeat > 1:
                with tc.For_i(0, loop_repeat, 1):
                    body()
            else:
                body()

    nc.compile()
    return nc


# ---------------------------------------------------------------------------
# Host side: prep constants (layout only), shard, run via PJRT (axon), unshard.
# ---------------------------------------------------------------------------

_CACHE = {}


def _prep_consts(centers, lengthscale, W_out, b_out):
    invl2 = 1.0 / (lengthscale.astype(np.float64) ** 2)

    csT = (centers.astype(np.float64) * invl2[None, :]).T     # [d, n]
    csT2 = np.empty((128, NT * NT * 128), dtype=ml_dtypes.bfloat16)
    for dc in range(NT):
        for nt in range(NT):
            csT2[:, (dc * NT + nt) * 128:(dc * NT + nt + 1) * 128] = \
                csT[dc * 128:(dc + 1) * 128, nt * 128:(nt + 1) * 128].astype(ml_dtypes.bfloat16)

    wTf = W_out.T.astype(np.float64)                          # [n, do]
    wT = np.empty((128, NT * NT * 128), dtype=ml_dtypes.bfloat16)
    for nt in range(NT):
        for dot in range(NT):
            wT[:, (nt * NT + dot) * 128:(nt * NT + dot + 1) * 128] = \
                wTf[nt * 128:(nt + 1) * 128, dot * 128:(dot + 1) * 128].astype(ml_dtypes.bfloat16)

    cn2 = np.sum(centers.astype(np.float64) ** 2 * invl2[None, :], axis=1)
    cnh = np.empty((128, NT), dtype=np.float32)
    for nt in range(NT):
        cnh[:, nt] = (-0.5 * cn2[nt * 128:(nt + 1) * 128]).astype(np.float32)

    bcol = np.empty((128, NT), dtype=np.float32)
    for dot in range(NT):
        bcol[:, dot] = b_out[dot * 128:(dot + 1) * 128].astype(np.float32)

    negiv = np.empty((128, NT), dtype=ml_dtypes.bfloat16)
    for dc in range(NT):
        negiv[:, dc] = (-0.5 * invl2[dc * 128:(dc + 1) * 128]).astype(ml_dtypes.bfloat16)

    onesb = np.ones((1, 128), dtype=ml_dtypes.bfloat16)
    return dict(csT2=csT2, wT=wT, cnh=cnh, bcol=bcol, negiv=negiv, onesb=onesb)


def _prep_x(x):
    """[B, L, D] f32 -> per-core [128, n_blocks*NT*MB] bf16 tiles (d on
    partitions, free = (mb, dc, m)). Pure layout prep."""
    x_flat = np.asarray(x, dtype=np.float32).reshape(BL, D)
    shards = []
    for c in range(N_CORES):
        xc = x_flat[c * M_CORE:(c + 1) * M_CORE]            # [4096, 512]
        xt = xc.T.reshape(NT, 128, N_BLOCKS, MB)            # (dc, p, mb, m)
        xt = xt.transpose(1, 2, 0, 3).reshape(128, N_BLOCKS * NT * MB)
        shards.append(np.ascontiguousarray(xt.astype(ml_dtypes.bfloat16)))
    return shards


def _get_runner(loop_repeat=1):
    key = ("runner", loop_repeat)
    if key in _CACHE:
        return _CACHE[key]

    nc = build_nc(loop_repeat=loop_repeat)

    import jax
    from jax.sharding import Mesh, PartitionSpec
    from jax.experimental.shard_map import shard_map
    from concourse import bass2jax
    from concourse import mybir as _mybir

    bass2jax.install_neuronx_cc_hook()

    partition_name = nc.partition_id_tensor.name if nc.partition_id_tensor else None
    in_names, out_names, out_avals, zero_shapes = [], [], [], []
    for alloc in nc.m.functions[0].allocations:
        if not isinstance(alloc, _mybir.MemoryLocationSet):
            continue
        name = alloc.memorylocations[0].name
        if alloc.kind == "ExternalInput":
            if name != partition_name:
                in_names.append(name)
        elif alloc.kind == "ExternalOutput":
            out_names.append(name)
            shape = tuple(alloc.tensor_shape)
            dtype = _mybir.dt.np(alloc.dtype)
            out_avals.append(jax.core.ShapedArray(shape, dtype))
            zero_shapes.append((shape, dtype))
    n_params = len(in_names)
    n_outs = len(out_avals)
    all_in_names = in_names + out_names
    if partition_name is not None:
        all_in_names = all_in_names + [partition_name]
    donate = tuple(range(n_params, n_params + n_outs))

    def _body(*args):
        operands = list(args)
        if partition_name is not None:
            operands.append(bass2jax.partition_id_tensor())
        outs = bass2jax._bass_exec_p.bind(
            *operands,
            out_avals=tuple(out_avals),
            in_names=tuple(all_in_names),
            out_names=tuple(out_names),
            lowering_input_output_aliases=(),
            sim_require_finite=True,
            sim_require_nnan=True,
            nc=nc,
        )
        return tuple(outs)

    devices = jax.devices()[:N_CORES]
    mesh = Mesh(np.asarray(devices), ("core",))
    in_specs = (PartitionSpec("core"),) * (n_params + n_outs)
    out_specs = (PartitionSpec("core"),) * n_outs
    sharded = jax.jit(
        shard_map(_body, mesh=mesh, in_specs=in_specs, out_specs=out_specs,
                  check_rep=False),
        donate_argnums=donate, keep_unused=True,
    )

    def run(in_maps):
        per_core = [[np.asarray(m[name]) for name in in_names] for m in in_maps]
        concat_in = [
            np.concatenate([per_core[c][i] for c in range(N_CORES)], axis=0)
            for i in range(n_params)
        ]
        concat_zeros = [
            np.zeros((N_CORES * s[0], *s[1:]), dt) for (s, dt) in zero_shapes
        ]
        out_arrs = sharded(*concat_in, *concat_zeros)
        return [
            {
                name: np.asarray(out_arrs[i]).reshape(N_CORES, *out_avals[i].shape)[c]
                for i, name in enumerate(out_names)
            }
            for c in range(N_CORES)
        ]

    run.in_names = in_names
    run.sharded = sharded
    run.nc = nc
    _CACHE[key] = run
    return run


def kernel(x, centers, lengthscale, W_out, b_out):
    x = np.asarray(x)
    centers = np.asarray(centers)
    lengthscale = np.asarray(lengthscale)
    W_out = np.asarray(W_out)
    b_out = np.asarray(b_out)

    consts = _prep_consts(centers, lengthscale, W_out, b_out)
    run = _get_runner()

    x_shards = _prep_x(x)
    in_maps = []
    for c in range(N_CORES):
        m = dict(consts)
        m["xt"] = x_shards[c]
        in_maps.append(m)

    results = run(in_maps)

    outs = []
    for c in range(N_CORES):
        yc = results[c]["y"]                       # [nb, 128(do%128), NT(dot), MB(m)]
        yc = yc.reshape(N_BLOCKS, 128, NT, MB).astype(np.float32)
        yc = yc.transpose(0, 3, 2, 1).reshape(M_CORE, D)
        outs.append(yc)
    out = np.concatenate(outs, axis=0).reshape(B, L, D)
    return out.astype(np.float32)


def build_for_sim():
    return build_nc()
